# revision 13
# baseline (speedup 1.0000x reference)
"""Masked dot-product attention on 8 Trainium2 NeuronCores.

Problem: B=32 heads of Q=K=2048, D=128, f32, boolean mask, softmax over K.
    out = softmax(where(mask, -1e6, Q@K^T/sqrt(D)), axis=-1) @ V

Strategy (per spec sharding hint): shard B across the 8 cores (4 heads each),
no cross-core communication.

Per-core kernel (all in "transposed" S^T = [k_partition, q_free] layout so the
P@V matmul needs no on-chip transposes):
  - host supplies Q^T, K^T ([d, q] / [d, k] layouts), V natural, and the
    keep-mask NM = (1 - mask)^T in two formats split by k-chunk:
      * A-chunks (PE-masked): fp8e4 bytes {0, 1.0}; an extra accumulating
        matmul with a 112*I fp8e5 stationary adds +112 to kept lanes, and the
        exp bias subtracts 112*SCALE, so masked lanes underflow to ~0
        (leakage exp(-9.9) ~ 5e-5 relative -- negligible).  1 B/elem of DMA,
        no elementwise mask op.  (fp8 matmuls cost the same PE cycles as
        fp16 -- the win is DMA bytes only.)
      * B-chunks (VectorE-masked): fp16 {0,1}; pm = p * nm on DVE in 2x mode.
    (GpSimd masking was tried and abandoned: Pool-engine compute contends
    with VectorE's SBUF port and slows every concurrent DVE op ~1.75x.)
  - S^T[k, qb] = K^T_chunk.T @ Q^T  (TensorE, fp16 in / f32 accumulate)
  - P^T = exp(S^T * 1/sqrt(D)) on ScalarE (no max-subtraction needed:
    scores ~ N(0,1), exp cannot overflow; masked lanes underflow to 0).
  - O^T[d, qb] += V_chunk.T(natural lhsT) @ P^T_chunk  (TensorE, fp16),
    software-pipelined PV_DEPTH=3 chunks behind the exp/mask chain; the last
    3 PV matmuls of each half are deferred into the next half AFTER its
    kc=0 S matmul, so ScalarE's exp stream never gaps at half boundaries.
  - denominator: two accumulator chains on VectorE with NON-in-place adds
    (chain step writes a fresh tile) so the deferred PV reads of the chain
    heads never create write-after-read stalls; chain heads are written
    directly by the first mask-mul / first A-chunk exp (no init copies).
    ones[128,128] @ chain tails broadcasts the k-sum to all partitions
    (TensorE); reciprocal_approx_fast on VectorE; O = O_un * r (VectorE).
  - each q-half's epilogue (denominator matmul, reciprocal, normalize) is
    deferred into the next half's kc=1 so it never stalls the PE queue.
  - masks are DMA'd in <=4-chunk slices so the first chunks of a half never
    wait on one monolithic transfer.
  - output written as O^T [d, q] fp16; host transposes/upcasts on unshard.
"""

import os
import sys
import numpy as np
from contextlib import ExitStack

for _p in ("/opt/trn_rl_repo", "/root/.axon_site",
           "/root/.axon_site/_ro/pypackages"):
    if _p not in sys.path:
        sys.path.append(_p)


def _ensure_axon_hooks_stub():
    """concourse imports antenv.axon_hooks when BASS_TRACE is set; this image
    may lack the module. Provide a no-op registry so tracing degrades
    gracefully instead of crashing."""
    try:
        import antenv.axon_hooks  # noqa: F401
        return
    except Exception:
        pass
    try:
        import types
        import antenv

        mod = types.ModuleType("antenv.axon_hooks")
        mod._hook = None
        mod.set_axon_ntff_profile_hook = lambda h: setattr(mod, "_hook", h)
        mod.get_axon_ntff_profile_hook = lambda: mod._hook
        sys.modules["antenv.axon_hooks"] = mod
        antenv.axon_hooks = mod
    except Exception:
        pass

# ---- problem constants (hardcoded per the self-containment contract) ----
B, Q, K, D = 32, 2048, 2048, 128
N_CORES = 8
BPC = B // N_CORES          # heads per core
KC = K // 128               # k chunks of 128 (partition dim of S^T)
QT_W = 1024                 # S^T psum tile width (2 psum banks)
NQT = Q // QT_W
SCALE = 1.0 / float(np.sqrt(D))

# chunks masked on the PE via the fp8 matmul (rest: VectorE fp16 mul)
A_KCS = (2, 5, 8, 11, 13, 15)
MASK_LAM = 112.0            # PE-mask magnitude; 112 = 1.75*2^6 exact in e5m2
FP8E5_LAM_BYTE = 0x57       # e5m2 encoding of 112.0
FP8E4_ONE_BYTE = 0x38       # e4m3 encoding of 1.0
PV_DEPTH = 3                # PV matmul pipelined this many chunks behind

_CACHED_NC = None
LAST_RESULTS = None  # BassKernelResults of the most recent run (for test.py)


def _build():
    import concourse.tile as tile
    from concourse import bacc, mybir

    FP16 = mybir.dt.float16
    F32 = mybir.dt.float32
    U8 = mybir.dt.uint8
    FP8E4 = mybir.dt.float8e4
    FP8E5 = mybir.dt.float8e5
    EXP = mybir.ActivationFunctionType.Exp

    nc = bacc.Bacc("TRN2", target_bir_lowering=False, debug=False,
                   enable_asserts=False, num_devices=N_CORES)

    nA = len(A_KCS)
    nB = KC - nA
    B_KCS = tuple(k for k in range(KC) if k not in A_KCS)
    a_pos = {kc: i for i, kc in enumerate(A_KCS)}
    b_pos = {kc: i for i, kc in enumerate(B_KCS)}

    qt_d = nc.dram_tensor("qt", [BPC, 128, Q], FP16, kind="ExternalInput").ap()
    kt_d = nc.dram_tensor("kt", [BPC, 128, K], FP16, kind="ExternalInput").ap()
    v_d = nc.dram_tensor("v", [BPC, K, D], FP16, kind="ExternalInput").ap()
    nm16_d = nc.dram_tensor("nm16", [BPC, nB * 128, Q], FP16,
                            kind="ExternalInput").ap()
    nm8_d = nc.dram_tensor("nm8", [BPC, nA * 128, Q], U8,
                           kind="ExternalInput").ap()
    negi8_d = nc.dram_tensor("negi8", [128, 128], U8, kind="ExternalInput").ap()
    out_d = nc.dram_tensor("out", [BPC, 128, Q], FP16, kind="ExternalOutput").ap()

    with tile.TileContext(nc) as tc, ExitStack() as ctx:
        consts = ctx.enter_context(tc.tile_pool(name="consts", bufs=1))
        io = ctx.enter_context(tc.tile_pool(name="io", bufs=3))
        nm16_pool = ctx.enter_context(tc.tile_pool(name="nm16", bufs=2))
        nm8_pool = ctx.enter_context(tc.tile_pool(name="nm8", bufs=2))
        p_pool = ctx.enter_context(tc.tile_pool(name="p", bufs=10))
        pm_pool = ctx.enter_context(tc.tile_pool(name="pm", bufs=10))
        ch_pool = ctx.enter_context(tc.tile_pool(name="ch", bufs=6))
        r_pool = ctx.enter_context(tc.tile_pool(name="r", bufs=2))
        ob_pool = ctx.enter_context(tc.tile_pool(name="ob", bufs=2))
        s_psum = ctx.enter_context(tc.tile_pool(name="sps", bufs=3, space="PSUM"))
        o_psum = ctx.enter_context(tc.tile_pool(name="ops", bufs=1, space="PSUM"))

        ones_sb = consts.tile([128, 128], FP16)
        nc.vector.memset(ones_sb, 1.0)
        negi8_sb = consts.tile([128, 128], U8)
        nc.sync.dma_start(out=negi8_sb, in_=negi8_d)
        bias_sb = consts.tile([128, 1], F32)
        nc.vector.memset(bias_sb, -MASK_LAM * SCALE)
        negi_f8 = negi8_sb.bitcast(FP8E5)

        pending_epi = None
        pending_pv = []  # last PV_DEPTH PV matmuls, deferred into next half

        def emit_epilogue(o_ps, acc, accg, ob_sb, h, b):
            # denominator + normalize + store; deferred into the next
            # q-half's kc=1 so these ops never stall the in-order PE queue
            l_ps = s_psum.tile([128, QT_W], F32, tag="s", name="l_ps")
            for j in range(QT_W // 512):
                jj = slice(j * 512, (j + 1) * 512)
                nc.tensor.matmul(l_ps[:, jj], ones_sb, acc[:, jj],
                                 start=True, stop=accg is None)
                if accg is not None:
                    nc.tensor.matmul(l_ps[:, jj], ones_sb, accg[:, jj],
                                     start=False, stop=True)
            r_sb = r_pool.tile([128, QT_W], F32, tag="r", name="r_sb")
            nc.vector.reciprocal_approx_fast(r_sb, l_ps)
            hq = slice(h * QT_W, (h + 1) * QT_W)
            nc.vector.tensor_mul(ob_sb[:, hq], o_ps, r_sb)
            # store each half's slice as soon as it is normalized, so the
            # final store at kernel end is only half a head
            nc.sync.dma_start(out=out_d[b][:, hq], in_=ob_sb[:, hq])

        def emit_mask_loads(b, h, split16=3):
            """Allocate + DMA the mask tiles for (b, h), nm16 in slices."""
            hq = slice(h * QT_W, (h + 1) * QT_W)
            nm16_sb = nm16_pool.tile([128, nB * QT_W], FP16, tag="nm16")
            edges = [round(nB * i / split16) for i in range(split16 + 1)]
            for c0, c1 in zip(edges, edges[1:]):
                if c1 == c0:
                    continue
                nc.sync.dma_start(
                    out=nm16_sb[:, c0 * QT_W:c1 * QT_W]
                    .rearrange("p (c q) -> p c q", c=c1 - c0),
                    in_=nm16_d[b][c0 * 128:c1 * 128, hq]
                    .rearrange("(c p) q -> p c q", p=128))
            nm8_sb = nm8_pool.tile([128, nA * QT_W], U8, tag="nm8")
            nc.sync.dma_start(
                out=nm8_sb.rearrange("p (c q) -> p c q", c=nA),
                in_=nm8_d[b][:, hq].rearrange("(c p) q -> p c q", p=128))
            return (nm16_sb, nm8_sb)

        mask_tiles = {}  # (b, h) -> tile pair, prefetched one half ahead

        def dma_v_slice(v_sb, b, k0, k1):
            nc.sync.dma_start(
                out=v_sb[:, k0 * D:k1 * D]
                .rearrange("p (kc d) -> p kc d", kc=k1 - k0),
                in_=v_d[b][k0 * 128:k1 * 128]
                .rearrange("(kc p) q -> p kc q", p=128),
            )

        for b in range(BPC):
            qt_sb = io.tile([128, Q], FP16, tag="qt")
            kt_sb = io.tile([128, K], FP16, tag="kt")
            v_sb = io.tile([128, KC * D], FP16, tag="v")
            ob_sb = ob_pool.tile([128, Q], FP16, tag="ob")
            if b == 0:
                # cold start: issue loads in the order the first chunks need
                # them -- tiny kt/qt head first so the first S matmuls fire
                # ASAP, then early masks (kc2 is PE-masked: nm8 before bulk),
                # then the rest interleaved ahead of its consumption point
                nc.sync.dma_start(out=kt_sb[:, 0:128], in_=kt_d[b][:, 0:128])
                nc.sync.dma_start(out=qt_sb[:, 0:QT_W], in_=qt_d[b][:, 0:QT_W])
                nc.sync.dma_start(out=kt_sb[:, 128:512], in_=kt_d[b][:, 128:512])
                mask_tiles[(0, 0)] = emit_mask_loads(0, 0, split16=5)
                nc.sync.dma_start(out=kt_sb[:, 512:1024],
                                  in_=kt_d[b][:, 512:1024])
                dma_v_slice(v_sb, b, 0, 6)
                nc.sync.dma_start(out=kt_sb[:, 1024:], in_=kt_d[b][:, 1024:])
                dma_v_slice(v_sb, b, 6, KC)
                nc.sync.dma_start(out=qt_sb[:, QT_W:], in_=qt_d[b][:, QT_W:])
            else:
                nc.sync.dma_start(out=kt_sb[:, 0:512], in_=kt_d[b][:, 0:512])
                nc.sync.dma_start(out=qt_sb[:, 0:QT_W], in_=qt_d[b][:, 0:QT_W])
                nc.sync.dma_start(out=kt_sb[:, 512:], in_=kt_d[b][:, 512:])
                nc.sync.dma_start(out=qt_sb[:, QT_W:], in_=qt_d[b][:, QT_W:])
                # V natural [K, D] -> [128 (k within chunk), KC*D]
                dma_v_slice(v_sb, b, 0, KC)

            for h in range(NQT):
                # o_ps is allocated lazily at its first PV write: the bufs=1
                # ring slot is still being written (deferred PV tail) and
                # read (deferred epilogue) for the PREVIOUS half until kc=1
                o_ps = None
                nm16_sb, nm8_sb = mask_tiles.pop((b, h))
                nm8_f8 = nm8_sb.bitcast(FP8E4)

                pv_queue = []       # (vchunk, pm, kc) pending PV matmuls
                acc = None          # B-chain tail (fresh tile per add)
                accg = None         # A-chain tail

                def flush_pv(q, target_ps, final):
                    for qi, (pv_vc, pv_pm, pv_kc) in enumerate(q):
                        stop = final and qi == len(q) - 1
                        for j in range(QT_W // 512):
                            jj = slice(j * 512, (j + 1) * 512)
                            nc.tensor.matmul(target_ps[:, jj], pv_vc,
                                             pv_pm[:, jj],
                                             start=(pv_kc == 0), stop=stop)

                for kc in range(KC):
                    is_a = kc in a_pos

                    kchunk = kt_sb[:, kc * 128:(kc + 1) * 128]
                    vchunk = v_sb[:, kc * D:(kc + 1) * D]
                    s_ps = s_psum.tile([128, QT_W], F32, tag="s")
                    for j in range(QT_W // 512):
                        jj = slice(j * 512, (j + 1) * 512)
                        nc.tensor.matmul(s_ps[:, jj], kchunk,
                                         qt_sb[:, h * QT_W + j * 512:
                                               h * QT_W + (j + 1) * 512],
                                         start=True, stop=not is_a)
                        if is_a:
                            a0 = a_pos[kc] * QT_W
                            nc.tensor.matmul(
                                s_ps[:, jj], negi_f8,
                                nm8_f8[:, a0 + j * 512:a0 + (j + 1) * 512],
                                start=False, stop=True)

                    if kc == 1:
                        # previous half's PV tail, after this half's first
                        # TWO S matmuls so ScalarE's exp stream never gaps
                        # across the boundary
                        if pending_pv:
                            flush_pv(pending_pv, prev_o_ps, final=True)
                            pending_pv = []
                        if pending_epi is not None:
                            emit_epilogue(*pending_epi)
                            pending_epi = None
                    if kc == 2:
                        # prefetch next half's masks
                        nb, nh = (b, h + 1) if h + 1 < NQT else (b + 1, 0)
                        if nb < BPC:
                            mask_tiles[(nb, nh)] = emit_mask_loads(nb, nh)

                    if is_a:
                        # first A-chunk's exp writes the chain head directly
                        if accg is None:
                            p_sb = ch_pool.tile([128, QT_W], FP16, tag="ch",
                                                name="accg0")
                        else:
                            p_sb = p_pool.tile([128, QT_W], FP16, tag="p")
                        nc.scalar.activation(p_sb, s_ps, EXP, scale=SCALE,
                                             bias=bias_sb[:, 0:1])
                        pm = p_sb
                        if accg is None:
                            accg = p_sb
                        else:
                            t = ch_pool.tile([128, QT_W], FP16, tag="ch")
                            nc.vector.tensor_add(t, accg, pm)
                            accg = t
                    else:
                        p_sb = p_pool.tile([128, QT_W], FP16, tag="p")
                        nc.scalar.activation(p_sb, s_ps, EXP, scale=SCALE)
                        # first B-chunk's mul writes the chain head directly
                        pm = ch_pool.tile([128, QT_W], FP16, tag="ch",
                                          name="acc0") if acc is None else \
                            pm_pool.tile([128, QT_W], FP16, tag="pm")
                        b0 = b_pos[kc] * QT_W
                        nc.vector.tensor_mul(pm, p_sb, nm16_sb[:, b0:b0 + QT_W])
                        if acc is None:
                            acc = pm
                        else:
                            t = ch_pool.tile([128, QT_W], FP16, tag="ch")
                            nc.vector.tensor_add(t, acc, pm)
                            acc = t

                    pv_queue.append((vchunk, pm, kc))
                    # in the very last half, drain eagerly (depth 1) so the
                    # kernel tail is short; elsewhere keep PV_DEPTH of slack
                    depth = 1 if (b == BPC - 1 and h == NQT - 1
                                  and kc >= KC - PV_DEPTH) else PV_DEPTH
                    while len(pv_queue) > depth:
                        if o_ps is None:
                            o_ps = o_psum.tile([128, QT_W], F32, tag="o",
                                               name=f"o{h}")
                        flush_pv(pv_queue[:1], o_ps, final=False)
                        pv_queue = pv_queue[1:]

                last_half = b == BPC - 1 and h == NQT - 1
                if last_half:
                    flush_pv(pv_queue, o_ps, final=True)
                    pv_queue = []
                pending_pv = pv_queue
                prev_o_ps = o_ps
                pending_epi = (o_ps, acc, accg, ob_sb, h, b)

        if pending_epi is not None:
            emit_epilogue(*pending_epi)

    nc.compile()
    return nc


def _get_nc():
    global _CACHED_NC
    if _CACHED_NC is None:
        _CACHED_NC = _build()
    return _CACHED_NC


def kernel(queries, keys, values, mask_idx, **_unused):
    global LAST_RESULTS
    _ensure_axon_hooks_stub()
    from concourse import bass_utils

    queries = np.asarray(queries, dtype=np.float32)
    keys = np.asarray(keys, dtype=np.float32)
    values = np.asarray(values, dtype=np.float32)
    mask_idx = np.asarray(mask_idx)

    # host-side shard + reformat (layout only; no attention math on host)
    qt = np.ascontiguousarray(
        queries.reshape(N_CORES, BPC, Q, D).transpose(0, 1, 3, 2)).astype(
        np.float16)
    kt = np.ascontiguousarray(
        keys.reshape(N_CORES, BPC, K, D).transpose(0, 1, 3, 2)).astype(
        np.float16)
    v = values.reshape(N_CORES, BPC, K, D).astype(np.float16)
    # keep-mask, transposed to [K, Q] per head, split into the 2 formats
    nmt = np.ascontiguousarray(
        (~mask_idx.astype(bool)).reshape(N_CORES, BPC, Q, K)
        .transpose(0, 1, 3, 2))
    kcs = np.arange(K) // 128
    a_rows = np.isin(kcs, A_KCS)
    nm16 = np.ascontiguousarray(nmt[:, :, ~a_rows, :]).astype(np.float16)
    nm8 = (np.ascontiguousarray(nmt[:, :, a_rows, :]).astype(np.uint8)
           * np.uint8(FP8E4_ONE_BYTE))
    negi8 = (np.eye(128) * FP8E5_LAM_BYTE).astype(np.uint8)

    in_maps = [
        {"qt": qt[c], "kt": kt[c], "v": np.ascontiguousarray(v[c]),
         "nm16": nm16[c], "nm8": nm8[c], "negi8": negi8}
        for c in range(N_CORES)
    ]

    nc = _get_nc()
    res = bass_utils.run_bass_kernel_spmd(nc, in_maps, core_ids=list(range(N_CORES)))
    LAST_RESULTS = res

    # gather + unshard: out is O^T [BPC, d, q] per core -> [B, Q, D]
    ot = np.stack([res.results[c]["out"] for c in range(N_CORES)])
    return np.ascontiguousarray(
        ot.transpose(0, 1, 3, 2).reshape(B, Q, D)).astype(np.float32)


# revision 20
# speedup vs baseline: 1.0209x; 1.0209x over previous
"""Masked dot-product attention on 8 Trainium2 NeuronCores.

Problem: B=32 heads of Q=K=2048, D=128, f32, boolean mask, softmax over K.
    out = softmax(where(mask, -1e6, Q@K^T/sqrt(D)), axis=-1) @ V

Strategy (per spec sharding hint): shard B across the 8 cores (4 heads each),
no cross-core communication.

Per-core kernel (all in "transposed" S^T = [k_partition, q_free] layout so the
P@V matmul needs no on-chip transposes):
  - host supplies Q^T, K^T ([d, q] / [d, k] layouts), V natural, and the
    keep-mask NM = (1 - mask)^T in two formats split by k-chunk:
      * A-chunks (PE-masked): fp8e4 bytes {0, 1.0}; an extra accumulating
        matmul with a 112*I fp8e5 stationary adds +112 to kept lanes, and the
        exp bias subtracts 112*SCALE, so masked lanes underflow to ~0
        (leakage exp(-9.9) ~ 5e-5 relative -- negligible).  1 B/elem of DMA,
        no elementwise mask op.  (fp8 matmuls cost the same PE cycles as
        fp16 -- the win is DMA bytes only.)
      * B-chunks (VectorE-masked): fp16 {0,1}; pm = p * nm on DVE in 2x mode.
    (GpSimd masking was tried and abandoned: Pool-engine compute contends
    with VectorE's SBUF port and slows every concurrent DVE op ~1.75x.)
  - S^T[k, qb] = K^T_chunk.T @ Q^T  (TensorE, fp16 in / f32 accumulate)
  - P^T = exp(S^T * 1/sqrt(D)) on ScalarE (no max-subtraction needed:
    scores ~ N(0,1), exp cannot overflow; masked lanes underflow to 0).
  - O^T[d, qb] += V_chunk.T(natural lhsT) @ P^T_chunk  (TensorE, fp16),
    software-pipelined PV_DEPTH=3 chunks behind the exp/mask chain; the last
    3 PV matmuls of each half are deferred into the next half AFTER its
    kc=0 S matmul, so ScalarE's exp stream never gaps at half boundaries.
  - denominator: two accumulator chains on VectorE with NON-in-place adds
    (chain step writes a fresh tile) so the deferred PV reads of the chain
    heads never create write-after-read stalls; chain heads are written
    directly by the first mask-mul / first A-chunk exp (no init copies).
    ones[128,128] @ chain tails broadcasts the k-sum to all partitions
    (TensorE); reciprocal_approx_fast on VectorE; O = O_un * r (VectorE).
  - each q-half's epilogue (denominator matmul, reciprocal, normalize) is
    deferred into the next half's kc=1 so it never stalls the PE queue.
  - masks are DMA'd in <=4-chunk slices so the first chunks of a half never
    wait on one monolithic transfer.
  - output written as O^T [d, q] fp16; host transposes/upcasts on unshard.
"""

import os
import sys
import numpy as np
from contextlib import ExitStack

for _p in ("/opt/trn_rl_repo", "/root/.axon_site",
           "/root/.axon_site/_ro/pypackages"):
    if _p not in sys.path:
        sys.path.append(_p)


def _ensure_axon_hooks_stub():
    """concourse imports antenv.axon_hooks when BASS_TRACE is set; this image
    may lack the module. Provide a no-op registry so tracing degrades
    gracefully instead of crashing."""
    try:
        import antenv.axon_hooks  # noqa: F401
        return
    except Exception:
        pass
    try:
        import types
        import antenv

        mod = types.ModuleType("antenv.axon_hooks")
        mod._hook = None
        mod.set_axon_ntff_profile_hook = lambda h: setattr(mod, "_hook", h)
        mod.get_axon_ntff_profile_hook = lambda: mod._hook
        sys.modules["antenv.axon_hooks"] = mod
        antenv.axon_hooks = mod
    except Exception:
        pass

# ---- problem constants (hardcoded per the self-containment contract) ----
B, Q, K, D = 32, 2048, 2048, 128
N_CORES = 8
BPC = B // N_CORES          # heads per core
KC = K // 128               # k chunks of 128 (partition dim of S^T)
QT_W = 1024                 # S^T psum tile width (2 psum banks)
NQT = Q // QT_W
SCALE = 1.0 / float(np.sqrt(D))

# chunks masked on the PE via the fp8 matmul (rest: VectorE fp16 mul)
A_KCS = (2, 5, 8, 11, 13, 15)
MASK_LAM = 112.0            # PE-mask magnitude; 112 = 1.75*2^6 exact in e5m2
FP8E5_LAM_BYTE = 0x57       # e5m2 encoding of 112.0
FP8E4_ONE_BYTE = 0x38       # e4m3 encoding of 1.0
PV_DEPTH = 6                # PV matmul pipelined this many chunks behind
EPI_KC = 6                  # previous half's epilogue emitted at this kc

_CACHED_NC = None
LAST_RESULTS = None  # BassKernelResults of the most recent run (for test.py)


def _build():
    import concourse.tile as tile
    from concourse import bacc, mybir

    FP16 = mybir.dt.float16
    F32 = mybir.dt.float32
    U8 = mybir.dt.uint8
    FP8E4 = mybir.dt.float8e4
    FP8E5 = mybir.dt.float8e5
    EXP = mybir.ActivationFunctionType.Exp

    nc = bacc.Bacc("TRN2", target_bir_lowering=False, debug=False,
                   enable_asserts=False, num_devices=N_CORES)

    nA = len(A_KCS)
    nB = KC - nA
    B_KCS = tuple(k for k in range(KC) if k not in A_KCS)
    a_pos = {kc: i for i, kc in enumerate(A_KCS)}
    b_pos = {kc: i for i, kc in enumerate(B_KCS)}

    qt_d = nc.dram_tensor("qt", [BPC, 128, Q], FP16, kind="ExternalInput").ap()
    kt_d = nc.dram_tensor("kt", [BPC, 128, K], FP16, kind="ExternalInput").ap()
    v_d = nc.dram_tensor("v", [BPC, K, D], FP16, kind="ExternalInput").ap()
    nm16_d = nc.dram_tensor("nm16", [BPC, nB * 128, Q], FP16,
                            kind="ExternalInput").ap()
    nm8_d = nc.dram_tensor("nm8", [BPC, nA * 128, Q], U8,
                           kind="ExternalInput").ap()
    negi8_d = nc.dram_tensor("negi8", [128, 128], U8, kind="ExternalInput").ap()
    out_d = nc.dram_tensor("out", [BPC, 128, Q], FP16, kind="ExternalOutput").ap()

    with tile.TileContext(nc) as tc, ExitStack() as ctx:
        consts = ctx.enter_context(tc.tile_pool(name="consts", bufs=1))
        io = ctx.enter_context(tc.tile_pool(name="io", bufs=3))
        nm16_pool = ctx.enter_context(tc.tile_pool(name="nm16", bufs=2))
        nm8_pool = ctx.enter_context(tc.tile_pool(name="nm8", bufs=2))
        p_pool = ctx.enter_context(tc.tile_pool(name="p", bufs=10))
        pm_pool = ctx.enter_context(tc.tile_pool(name="pm", bufs=10))
        ch_pool = ctx.enter_context(tc.tile_pool(name="ch", bufs=6))
        r_pool = ctx.enter_context(tc.tile_pool(name="r", bufs=2))
        ob_pool = ctx.enter_context(tc.tile_pool(name="ob", bufs=2))
        s_psum = ctx.enter_context(tc.tile_pool(name="sps", bufs=3, space="PSUM"))
        o_psum = ctx.enter_context(tc.tile_pool(name="ops", bufs=1, space="PSUM"))

        ones_sb = consts.tile([128, 128], FP16)
        nc.vector.memset(ones_sb, 1.0)
        negi8_sb = consts.tile([128, 128], U8)
        nc.sync.dma_start(out=negi8_sb, in_=negi8_d)
        bias_sb = consts.tile([128, 1], F32)
        nc.vector.memset(bias_sb, -MASK_LAM * SCALE)
        negi_f8 = negi8_sb.bitcast(FP8E5)

        pending_epi = None
        pending_pv = []  # last PV_DEPTH PV matmuls, deferred into next half

        def emit_epilogue(o_ps, acc, accg, ob_sb, h, b):
            # denominator + normalize + store; deferred into the next
            # q-half's kc=1 so these ops never stall the in-order PE queue
            l_ps = s_psum.tile([128, QT_W], F32, tag="s", name="l_ps")
            for j in range(QT_W // 512):
                jj = slice(j * 512, (j + 1) * 512)
                nc.tensor.matmul(l_ps[:, jj], ones_sb, acc[:, jj],
                                 start=True, stop=accg is None)
                if accg is not None:
                    nc.tensor.matmul(l_ps[:, jj], ones_sb, accg[:, jj],
                                     start=False, stop=True)
            r_sb = r_pool.tile([128, QT_W], F32, tag="r", name="r_sb")
            nc.vector.reciprocal_approx_fast(r_sb, l_ps)
            hq = slice(h * QT_W, (h + 1) * QT_W)
            nc.vector.tensor_mul(ob_sb[:, hq], o_ps, r_sb)
            # store each half's slice as soon as it is normalized, so the
            # final store at kernel end is only half a head
            nc.sync.dma_start(out=out_d[b][:, hq], in_=ob_sb[:, hq])

        def emit_nm16_slice(nm16_sb, b, h, c0, c1):
            hq = slice(h * QT_W, (h + 1) * QT_W)
            nc.sync.dma_start(
                out=nm16_sb[:, c0 * QT_W:c1 * QT_W]
                .rearrange("p (c q) -> p c q", c=c1 - c0),
                in_=nm16_d[b][c0 * 128:c1 * 128, hq]
                .rearrange("(c p) q -> p c q", p=128))

        def emit_nm8(nm8_sb, b, h):
            hq = slice(h * QT_W, (h + 1) * QT_W)
            nc.sync.dma_start(
                out=nm8_sb.rearrange("p (c q) -> p c q", c=nA),
                in_=nm8_d[b][:, hq].rearrange("(c p) q -> p c q", p=128))

        def emit_mask_loads(b, h, split16=3):
            """Allocate + DMA the mask tiles for (b, h), nm16 in slices."""
            nm16_sb = nm16_pool.tile([128, nB * QT_W], FP16, tag="nm16")
            edges = [round(nB * i / split16) for i in range(split16 + 1)]
            for c0, c1 in zip(edges, edges[1:]):
                if c1 > c0:
                    emit_nm16_slice(nm16_sb, b, h, c0, c1)
            nm8_sb = nm8_pool.tile([128, nA * QT_W], U8, tag="nm8")
            emit_nm8(nm8_sb, b, h)
            return (nm16_sb, nm8_sb)

        mask_tiles = {}  # (b, h) -> tile pair, prefetched one half ahead
        pv_queue = []    # (vchunk, pm, kc, (b,h)) -- persists across halves
        prev_o_ps = None

        def dma_v_slice(v_sb, b, k0, k1):
            nc.sync.dma_start(
                out=v_sb[:, k0 * D:k1 * D]
                .rearrange("p (kc d) -> p kc d", kc=k1 - k0),
                in_=v_d[b][k0 * 128:k1 * 128]
                .rearrange("(kc p) q -> p kc q", p=128),
            )

        for b in range(BPC):
            qt_sb = io.tile([128, Q], FP16, tag="qt")
            kt_sb = io.tile([128, K], FP16, tag="kt")
            v_sb = io.tile([128, KC * D], FP16, tag="v")
            ob_sb = ob_pool.tile([128, Q], FP16, tag="ob")
            if b == 0:
                # cold start: issue loads in the order the first chunks need
                # them -- kt/qt head first so the first S matmuls fire ASAP,
                # one early mask slice (and nm8: kc2 is PE-masked), then the
                # bulk, with the later mask slices JIT behind it
                nm16_sb0 = nm16_pool.tile([128, nB * QT_W], FP16, tag="nm16")
                nm8_sb0 = nm8_pool.tile([128, nA * QT_W], U8, tag="nm8")
                nc.sync.dma_start(out=kt_sb[:, 0:512], in_=kt_d[b][:, 0:512])
                nc.sync.dma_start(out=qt_sb[:, 0:QT_W], in_=qt_d[b][:, 0:QT_W])
                emit_nm16_slice(nm16_sb0, 0, 0, 0, 3)
                emit_nm8(nm8_sb0, 0, 0)
                nc.sync.dma_start(out=kt_sb[:, 512:], in_=kt_d[b][:, 512:])
                dma_v_slice(v_sb, b, 0, 6)
                emit_nm16_slice(nm16_sb0, 0, 0, 3, 6)
                nc.sync.dma_start(out=qt_sb[:, QT_W:], in_=qt_d[b][:, QT_W:])
                dma_v_slice(v_sb, b, 6, 11)
                emit_nm16_slice(nm16_sb0, 0, 0, 6, 8)
                dma_v_slice(v_sb, b, 11, KC)
                emit_nm16_slice(nm16_sb0, 0, 0, 8, nB)
                mask_tiles[(0, 0)] = (nm16_sb0, nm8_sb0)
            else:
                nc.sync.dma_start(out=kt_sb[:, 0:512], in_=kt_d[b][:, 0:512])
                nc.sync.dma_start(out=qt_sb[:, 0:QT_W], in_=qt_d[b][:, 0:QT_W])
                nc.sync.dma_start(out=kt_sb[:, 512:], in_=kt_d[b][:, 512:])
                nc.sync.dma_start(out=qt_sb[:, QT_W:], in_=qt_d[b][:, QT_W:])
                # V natural [K, D] -> [128 (k within chunk), KC*D]
                dma_v_slice(v_sb, b, 0, KC)

            for h in range(NQT):
                # o_ps is allocated lazily at this half's first PV pop
                # (kc=EPI_KC), right after the previous half's normalize has
                # been emitted -- the bufs=1 ring slot is busy until then
                o_ps = None
                nm16_sb, nm8_sb = mask_tiles.pop((b, h))
                nm8_f8 = nm8_sb.bitcast(FP8E4)

                acc = None          # B-chain tail (fresh tile per add)
                accg = None         # A-chain tail
                last_half = b == BPC - 1 and h == NQT - 1

                def pop_pv():
                    nonlocal o_ps, pv_queue
                    pv_vc, pv_pm, pv_kc, pv_h = pv_queue[0]
                    pv_queue = pv_queue[1:]
                    if pv_h == (b, h):
                        if o_ps is None:
                            o_ps = o_psum.tile([128, QT_W], F32, tag="o",
                                               name=f"o{h}")
                        target = o_ps
                    else:
                        target = prev_o_ps
                    for j in range(QT_W // 512):
                        jj = slice(j * 512, (j + 1) * 512)
                        nc.tensor.matmul(target[:, jj], pv_vc, pv_pm[:, jj],
                                         start=(pv_kc == 0),
                                         stop=(pv_kc == KC - 1))

                for kc in range(KC):
                    is_a = kc in a_pos

                    kchunk = kt_sb[:, kc * 128:(kc + 1) * 128]
                    vchunk = v_sb[:, kc * D:(kc + 1) * D]
                    s_ps = s_psum.tile([128, QT_W], F32, tag="s")
                    for j in range(QT_W // 512):
                        jj = slice(j * 512, (j + 1) * 512)
                        nc.tensor.matmul(s_ps[:, jj], kchunk,
                                         qt_sb[:, h * QT_W + j * 512:
                                               h * QT_W + (j + 1) * 512],
                                         start=True, stop=not is_a)
                        if is_a:
                            a0 = a_pos[kc] * QT_W
                            nc.tensor.matmul(
                                s_ps[:, jj], negi_f8,
                                nm8_f8[:, a0 + j * 512:a0 + (j + 1) * 512],
                                start=False, stop=True)

                    if kc == EPI_KC and pending_epi is not None:
                        # previous half's epilogue: by now its PV tail has
                        # fully popped (uniform one pop per kc), and the l
                        # tile displaces an s-ring slot whose exp is done
                        emit_epilogue(*pending_epi)
                        pending_epi = None
                    if kc == (8 if (b, h) == (0, 0) else 2):
                        # prefetch next half's masks (later for the cold
                        # start so they queue behind b0's own bulk loads)
                        nb, nh = (b, h + 1) if h + 1 < NQT else (b + 1, 0)
                        if nb < BPC:
                            mask_tiles[(nb, nh)] = emit_mask_loads(nb, nh)

                    if is_a:
                        # first A-chunk's exp writes the chain head directly
                        if accg is None:
                            p_sb = ch_pool.tile([128, QT_W], FP16, tag="ch",
                                                name="accg0")
                        else:
                            p_sb = p_pool.tile([128, QT_W], FP16, tag="p")
                        nc.scalar.activation(p_sb, s_ps, EXP, scale=SCALE,
                                             bias=bias_sb[:, 0:1])
                        pm = p_sb
                        if accg is None:
                            accg = p_sb
                        else:
                            t = ch_pool.tile([128, QT_W], FP16, tag="ch")
                            nc.vector.tensor_add(t, accg, pm)
                            accg = t
                    else:
                        p_sb = p_pool.tile([128, QT_W], FP16, tag="p")
                        nc.scalar.activation(p_sb, s_ps, EXP, scale=SCALE)
                        # first B-chunk's mul writes the chain head directly
                        pm = ch_pool.tile([128, QT_W], FP16, tag="ch",
                                          name="acc0") if acc is None else \
                            pm_pool.tile([128, QT_W], FP16, tag="pm")
                        b0 = b_pos[kc] * QT_W
                        nc.vector.tensor_mul(pm, p_sb, nm16_sb[:, b0:b0 + QT_W])
                        if acc is None:
                            acc = pm
                        else:
                            t = ch_pool.tile([128, QT_W], FP16, tag="ch")
                            nc.vector.tensor_add(t, acc, pm)
                            acc = t

                    pv_queue.append((vchunk, pm, kc, (b, h)))
                    # uniform software pipeline: one PV pop per kc (the queue
                    # carries the previous half's 6-entry tail across the
                    # boundary, draining at kc0-5, BEFORE the kc6 epilogue
                    # reads that half's o accumulation); in the last half,
                    # drain progressively so the kernel tail is short
                    depth = PV_DEPTH
                    if last_half:
                        depth = max(1, PV_DEPTH - max(0, kc - 8))
                    while len(pv_queue) > depth:
                        pop_pv()

                if last_half:
                    while pv_queue:
                        pop_pv()
                prev_o_ps = o_ps
                pending_epi = (o_ps, acc, accg, ob_sb, h, b)

        if pending_epi is not None:
            emit_epilogue(*pending_epi)

    nc.compile()
    return nc


def _get_nc():
    global _CACHED_NC
    if _CACHED_NC is None:
        _CACHED_NC = _build()
    return _CACHED_NC


def kernel(queries, keys, values, mask_idx, **_unused):
    global LAST_RESULTS
    _ensure_axon_hooks_stub()
    from concourse import bass_utils

    queries = np.asarray(queries, dtype=np.float32)
    keys = np.asarray(keys, dtype=np.float32)
    values = np.asarray(values, dtype=np.float32)
    mask_idx = np.asarray(mask_idx)

    # host-side shard + reformat (layout only; no attention math on host)
    qt = np.ascontiguousarray(
        queries.reshape(N_CORES, BPC, Q, D).transpose(0, 1, 3, 2)).astype(
        np.float16)
    kt = np.ascontiguousarray(
        keys.reshape(N_CORES, BPC, K, D).transpose(0, 1, 3, 2)).astype(
        np.float16)
    v = values.reshape(N_CORES, BPC, K, D).astype(np.float16)
    # keep-mask, transposed to [K, Q] per head, split into the 2 formats
    nmt = np.ascontiguousarray(
        (~mask_idx.astype(bool)).reshape(N_CORES, BPC, Q, K)
        .transpose(0, 1, 3, 2))
    kcs = np.arange(K) // 128
    a_rows = np.isin(kcs, A_KCS)
    nm16 = np.ascontiguousarray(nmt[:, :, ~a_rows, :]).astype(np.float16)
    nm8 = (np.ascontiguousarray(nmt[:, :, a_rows, :]).astype(np.uint8)
           * np.uint8(FP8E4_ONE_BYTE))
    negi8 = (np.eye(128) * FP8E5_LAM_BYTE).astype(np.uint8)

    in_maps = [
        {"qt": qt[c], "kt": kt[c], "v": np.ascontiguousarray(v[c]),
         "nm16": nm16[c], "nm8": nm8[c], "negi8": negi8}
        for c in range(N_CORES)
    ]

    nc = _get_nc()
    res = bass_utils.run_bass_kernel_spmd(nc, in_maps, core_ids=list(range(N_CORES)))
    LAST_RESULTS = res

    # gather + unshard: out is O^T [BPC, d, q] per core -> [B, Q, D]
    ot = np.stack([res.results[c]["out"] for c in range(N_CORES)])
    return np.ascontiguousarray(
        ot.transpose(0, 1, 3, 2).reshape(B, Q, D)).astype(np.float32)


# revision 22
# speedup vs baseline: 1.0289x; 1.0078x over previous
"""Masked dot-product attention on 8 Trainium2 NeuronCores.

Problem: B=32 heads of Q=K=2048, D=128, f32, boolean mask, softmax over K.
    out = softmax(where(mask, -1e6, Q@K^T/sqrt(D)), axis=-1) @ V

Strategy (per spec sharding hint): shard B across the 8 cores (4 heads each),
no cross-core communication.

Per-core kernel (all in "transposed" S^T = [k_partition, q_free] layout so the
P@V matmul needs no on-chip transposes):
  - host supplies Q^T, K^T ([d, q] / [d, k] layouts), V natural, and the
    keep-mask NM = (1 - mask)^T in two formats split by k-chunk:
      * A-chunks (PE-masked): fp8e4 bytes {0, 1.0}; an extra accumulating
        matmul with a 112*I fp8e5 stationary adds +112 to kept lanes, and the
        exp bias subtracts 112*SCALE, so masked lanes underflow to ~0
        (leakage exp(-9.9) ~ 5e-5 relative -- negligible).  1 B/elem of DMA,
        no elementwise mask op.  (fp8 matmuls cost the same PE cycles as
        fp16 -- the win is DMA bytes only.)
      * B-chunks (VectorE-masked): fp16 {0,1}; pm = p * nm on DVE in 2x mode.
    (GpSimd masking was tried and abandoned: Pool-engine compute contends
    with VectorE's SBUF port and slows every concurrent DVE op ~1.75x.)
  - S^T[k, qb] = K^T_chunk.T @ Q^T  (TensorE, fp16 in / f32 accumulate)
  - P^T = exp(S^T * 1/sqrt(D)) on ScalarE (no max-subtraction needed:
    scores ~ N(0,1), exp cannot overflow; masked lanes underflow to 0).
  - O^T[d, qb] += V_chunk.T(natural lhsT) @ P^T_chunk  (TensorE, fp16),
    software-pipelined PV_DEPTH=3 chunks behind the exp/mask chain; the last
    3 PV matmuls of each half are deferred into the next half AFTER its
    kc=0 S matmul, so ScalarE's exp stream never gaps at half boundaries.
  - denominator: two accumulator chains on VectorE with NON-in-place adds
    (chain step writes a fresh tile) so the deferred PV reads of the chain
    heads never create write-after-read stalls; chain heads are written
    directly by the first mask-mul / first A-chunk exp (no init copies).
    ones[128,128] @ chain tails broadcasts the k-sum to all partitions
    (TensorE); reciprocal_approx_fast on VectorE; O = O_un * r (VectorE).
  - each q-half's epilogue (denominator matmul, reciprocal, normalize) is
    deferred into the next half's kc=1 so it never stalls the PE queue.
  - masks are DMA'd in <=4-chunk slices so the first chunks of a half never
    wait on one monolithic transfer.
  - output written as O^T [d, q] fp16; host transposes/upcasts on unshard.
"""

import os
import sys
import numpy as np
from contextlib import ExitStack

for _p in ("/opt/trn_rl_repo", "/root/.axon_site",
           "/root/.axon_site/_ro/pypackages"):
    if _p not in sys.path:
        sys.path.append(_p)


def _ensure_axon_hooks_stub():
    """concourse imports antenv.axon_hooks when BASS_TRACE is set; this image
    may lack the module. Provide a no-op registry so tracing degrades
    gracefully instead of crashing."""
    try:
        import antenv.axon_hooks  # noqa: F401
        return
    except Exception:
        pass
    try:
        import types
        import antenv

        mod = types.ModuleType("antenv.axon_hooks")
        mod._hook = None
        mod.set_axon_ntff_profile_hook = lambda h: setattr(mod, "_hook", h)
        mod.get_axon_ntff_profile_hook = lambda: mod._hook
        sys.modules["antenv.axon_hooks"] = mod
        antenv.axon_hooks = mod
    except Exception:
        pass

# ---- problem constants (hardcoded per the self-containment contract) ----
B, Q, K, D = 32, 2048, 2048, 128
N_CORES = 8
BPC = B // N_CORES          # heads per core
KC = K // 128               # k chunks of 128 (partition dim of S^T)
QT_W = 1024                 # S^T psum tile width (2 psum banks)
NQT = Q // QT_W
SCALE = 1.0 / float(np.sqrt(D))

# chunks masked on the PE via the fp8 matmul (rest: VectorE fp16 mul);
# 5 chunks balances TensorE (the pacer at 6) against VectorE (pacer at 4)
A_KCS = (2, 5, 8, 11, 15)
MASK_LAM = 112.0            # PE-mask magnitude; 112 = 1.75*2^6 exact in e5m2
FP8E5_LAM_BYTE = 0x57       # e5m2 encoding of 112.0
FP8E4_ONE_BYTE = 0x38       # e4m3 encoding of 1.0
PV_DEPTH = 6                # PV matmul pipelined this many chunks behind
EPI_KC = 6                  # previous half's epilogue emitted at this kc

_CACHED_NC = None
LAST_RESULTS = None  # BassKernelResults of the most recent run (for test.py)


def _build():
    import concourse.tile as tile
    from concourse import bacc, mybir

    FP16 = mybir.dt.float16
    F32 = mybir.dt.float32
    U8 = mybir.dt.uint8
    FP8E4 = mybir.dt.float8e4
    FP8E5 = mybir.dt.float8e5
    EXP = mybir.ActivationFunctionType.Exp

    nc = bacc.Bacc("TRN2", target_bir_lowering=False, debug=False,
                   enable_asserts=False, num_devices=N_CORES)

    nA = len(A_KCS)
    nB = KC - nA
    B_KCS = tuple(k for k in range(KC) if k not in A_KCS)
    a_pos = {kc: i for i, kc in enumerate(A_KCS)}
    b_pos = {kc: i for i, kc in enumerate(B_KCS)}

    qt_d = nc.dram_tensor("qt", [BPC, 128, Q], FP16, kind="ExternalInput").ap()
    kt_d = nc.dram_tensor("kt", [BPC, 128, K], FP16, kind="ExternalInput").ap()
    v_d = nc.dram_tensor("v", [BPC, K, D], FP16, kind="ExternalInput").ap()
    nm16_d = nc.dram_tensor("nm16", [BPC, nB * 128, Q], FP16,
                            kind="ExternalInput").ap()
    nm8_d = nc.dram_tensor("nm8", [BPC, nA * 128, Q], U8,
                           kind="ExternalInput").ap()
    negi8_d = nc.dram_tensor("negi8", [128, 128], U8, kind="ExternalInput").ap()
    out_d = nc.dram_tensor("out", [BPC, 128, Q], FP16, kind="ExternalOutput").ap()

    with tile.TileContext(nc) as tc, ExitStack() as ctx:
        consts = ctx.enter_context(tc.tile_pool(name="consts", bufs=1))
        io = ctx.enter_context(tc.tile_pool(name="io", bufs=3))
        nm16_pool = ctx.enter_context(tc.tile_pool(name="nm16", bufs=2))
        nm8_pool = ctx.enter_context(tc.tile_pool(name="nm8", bufs=2))
        p_pool = ctx.enter_context(tc.tile_pool(name="p", bufs=14))
        pm_pool = ctx.enter_context(tc.tile_pool(name="pm", bufs=12))
        ch_pool = ctx.enter_context(tc.tile_pool(name="ch", bufs=8))
        r_pool = ctx.enter_context(tc.tile_pool(name="r", bufs=2))
        ob_pool = ctx.enter_context(tc.tile_pool(name="ob", bufs=2))
        s_psum = ctx.enter_context(tc.tile_pool(name="sps", bufs=3, space="PSUM"))
        o_psum = ctx.enter_context(tc.tile_pool(name="ops", bufs=1, space="PSUM"))

        ones_sb = consts.tile([128, 128], FP16)
        nc.vector.memset(ones_sb, 1.0)
        negi8_sb = consts.tile([128, 128], U8)
        nc.sync.dma_start(out=negi8_sb, in_=negi8_d)
        bias_sb = consts.tile([128, 1], F32)
        nc.vector.memset(bias_sb, -MASK_LAM * SCALE)
        negi_f8 = negi8_sb.bitcast(FP8E5)

        pending_epi = None
        pending_pv = []  # last PV_DEPTH PV matmuls, deferred into next half

        def emit_epilogue(o_ps, acc, accg, ob_sb, h, b):
            # denominator + normalize + store; deferred into the next
            # q-half's kc=1 so these ops never stall the in-order PE queue
            l_ps = s_psum.tile([128, QT_W], F32, tag="s", name="l_ps")
            for j in range(QT_W // 512):
                jj = slice(j * 512, (j + 1) * 512)
                nc.tensor.matmul(l_ps[:, jj], ones_sb, acc[:, jj],
                                 start=True, stop=accg is None)
                if accg is not None:
                    nc.tensor.matmul(l_ps[:, jj], ones_sb, accg[:, jj],
                                     start=False, stop=True)
            r_sb = r_pool.tile([128, QT_W], F32, tag="r", name="r_sb")
            nc.vector.reciprocal_approx_fast(r_sb, l_ps)
            hq = slice(h * QT_W, (h + 1) * QT_W)
            nc.vector.tensor_mul(ob_sb[:, hq], o_ps, r_sb)
            # store each half's slice as soon as it is normalized, so the
            # final store at kernel end is only half a head
            nc.sync.dma_start(out=out_d[b][:, hq], in_=ob_sb[:, hq])

        def emit_nm16_slice(nm16_sb, b, h, c0, c1):
            hq = slice(h * QT_W, (h + 1) * QT_W)
            nc.sync.dma_start(
                out=nm16_sb[:, c0 * QT_W:c1 * QT_W]
                .rearrange("p (c q) -> p c q", c=c1 - c0),
                in_=nm16_d[b][c0 * 128:c1 * 128, hq]
                .rearrange("(c p) q -> p c q", p=128))

        def emit_nm8(nm8_sb, b, h):
            hq = slice(h * QT_W, (h + 1) * QT_W)
            nc.sync.dma_start(
                out=nm8_sb.rearrange("p (c q) -> p c q", c=nA),
                in_=nm8_d[b][:, hq].rearrange("(c p) q -> p c q", p=128))

        def emit_mask_loads(b, h, split16=3):
            """Allocate + DMA the mask tiles for (b, h), nm16 in slices."""
            nm16_sb = nm16_pool.tile([128, nB * QT_W], FP16, tag="nm16")
            edges = [round(nB * i / split16) for i in range(split16 + 1)]
            for c0, c1 in zip(edges, edges[1:]):
                if c1 > c0:
                    emit_nm16_slice(nm16_sb, b, h, c0, c1)
            nm8_sb = nm8_pool.tile([128, nA * QT_W], U8, tag="nm8")
            emit_nm8(nm8_sb, b, h)
            return (nm16_sb, nm8_sb)

        mask_tiles = {}  # (b, h) -> tile pair, prefetched one half ahead
        pv_queue = []    # (vchunk, pm, kc, (b,h)) -- persists across halves
        prev_o_ps = None

        def dma_v_slice(v_sb, b, k0, k1):
            nc.sync.dma_start(
                out=v_sb[:, k0 * D:k1 * D]
                .rearrange("p (kc d) -> p kc d", kc=k1 - k0),
                in_=v_d[b][k0 * 128:k1 * 128]
                .rearrange("(kc p) q -> p kc q", p=128),
            )

        for b in range(BPC):
            qt_sb = io.tile([128, Q], FP16, tag="qt")
            kt_sb = io.tile([128, K], FP16, tag="kt")
            v_sb = io.tile([128, KC * D], FP16, tag="v")
            ob_sb = ob_pool.tile([128, Q], FP16, tag="ob")
            if b == 0:
                # cold start: issue loads in the order the first chunks need
                # them -- kt/qt head first so the first S matmuls fire ASAP,
                # one early mask slice (and nm8: kc2 is PE-masked), then the
                # bulk, with the later mask slices JIT behind it
                nm16_sb0 = nm16_pool.tile([128, nB * QT_W], FP16, tag="nm16")
                nm8_sb0 = nm8_pool.tile([128, nA * QT_W], U8, tag="nm8")
                nc.sync.dma_start(out=kt_sb[:, 0:512], in_=kt_d[b][:, 0:512])
                nc.sync.dma_start(out=qt_sb[:, 0:QT_W], in_=qt_d[b][:, 0:QT_W])
                emit_nm16_slice(nm16_sb0, 0, 0, 0, 3)
                emit_nm8(nm8_sb0, 0, 0)
                nc.sync.dma_start(out=kt_sb[:, 512:], in_=kt_d[b][:, 512:])
                dma_v_slice(v_sb, b, 0, 6)
                emit_nm16_slice(nm16_sb0, 0, 0, 3, 6)
                nc.sync.dma_start(out=qt_sb[:, QT_W:], in_=qt_d[b][:, QT_W:])
                dma_v_slice(v_sb, b, 6, 11)
                emit_nm16_slice(nm16_sb0, 0, 0, 6, 8)
                dma_v_slice(v_sb, b, 11, KC)
                emit_nm16_slice(nm16_sb0, 0, 0, 8, nB)
                mask_tiles[(0, 0)] = (nm16_sb0, nm8_sb0)
            else:
                nc.sync.dma_start(out=kt_sb[:, 0:512], in_=kt_d[b][:, 0:512])
                nc.sync.dma_start(out=qt_sb[:, 0:QT_W], in_=qt_d[b][:, 0:QT_W])
                nc.sync.dma_start(out=kt_sb[:, 512:], in_=kt_d[b][:, 512:])
                nc.sync.dma_start(out=qt_sb[:, QT_W:], in_=qt_d[b][:, QT_W:])
                # V natural [K, D] -> [128 (k within chunk), KC*D]
                dma_v_slice(v_sb, b, 0, KC)

            for h in range(NQT):
                # o_ps is allocated lazily at this half's first PV pop
                # (kc=EPI_KC), right after the previous half's normalize has
                # been emitted -- the bufs=1 ring slot is busy until then
                o_ps = None
                nm16_sb, nm8_sb = mask_tiles.pop((b, h))
                nm8_f8 = nm8_sb.bitcast(FP8E4)

                acc = None          # B-chain tail (fresh tile per add)
                accg = None         # A-chain tail
                last_half = b == BPC - 1 and h == NQT - 1

                def pop_pv():
                    nonlocal o_ps, pv_queue
                    pv_vc, pv_pm, pv_kc, pv_h = pv_queue[0]
                    pv_queue = pv_queue[1:]
                    if pv_h == (b, h):
                        if o_ps is None:
                            o_ps = o_psum.tile([128, QT_W], F32, tag="o",
                                               name=f"o{h}")
                        target = o_ps
                    else:
                        target = prev_o_ps
                    for j in range(QT_W // 512):
                        jj = slice(j * 512, (j + 1) * 512)
                        nc.tensor.matmul(target[:, jj], pv_vc, pv_pm[:, jj],
                                         start=(pv_kc == 0),
                                         stop=(pv_kc == KC - 1))

                for kc in range(KC):
                    is_a = kc in a_pos

                    kchunk = kt_sb[:, kc * 128:(kc + 1) * 128]
                    vchunk = v_sb[:, kc * D:(kc + 1) * D]
                    s_ps = s_psum.tile([128, QT_W], F32, tag="s")
                    for j in range(QT_W // 512):
                        jj = slice(j * 512, (j + 1) * 512)
                        nc.tensor.matmul(s_ps[:, jj], kchunk,
                                         qt_sb[:, h * QT_W + j * 512:
                                               h * QT_W + (j + 1) * 512],
                                         start=True, stop=not is_a)
                        if is_a:
                            a0 = a_pos[kc] * QT_W
                            nc.tensor.matmul(
                                s_ps[:, jj], negi_f8,
                                nm8_f8[:, a0 + j * 512:a0 + (j + 1) * 512],
                                start=False, stop=True)

                    if kc == EPI_KC and pending_epi is not None:
                        # previous half's epilogue: by now its PV tail has
                        # fully popped (uniform one pop per kc), and the l
                        # tile displaces an s-ring slot whose exp is done
                        emit_epilogue(*pending_epi)
                        pending_epi = None
                    if kc == (8 if (b, h) == (0, 0) else 2):
                        # prefetch next half's masks (later for the cold
                        # start so they queue behind b0's own bulk loads)
                        nb, nh = (b, h + 1) if h + 1 < NQT else (b + 1, 0)
                        if nb < BPC:
                            mask_tiles[(nb, nh)] = emit_mask_loads(nb, nh)

                    if is_a:
                        # first A-chunk's exp writes the chain head directly
                        if accg is None:
                            p_sb = ch_pool.tile([128, QT_W], FP16, tag="ch",
                                                name="accg0")
                        else:
                            p_sb = p_pool.tile([128, QT_W], FP16, tag="p")
                        nc.scalar.activation(p_sb, s_ps, EXP, scale=SCALE,
                                             bias=bias_sb[:, 0:1])
                        pm = p_sb
                        if accg is None:
                            accg = p_sb
                        else:
                            t = ch_pool.tile([128, QT_W], FP16, tag="ch")
                            nc.vector.tensor_add(t, accg, pm)
                            accg = t
                    else:
                        p_sb = p_pool.tile([128, QT_W], FP16, tag="p")
                        nc.scalar.activation(p_sb, s_ps, EXP, scale=SCALE)
                        # first B-chunk's mul writes the chain head directly
                        pm = ch_pool.tile([128, QT_W], FP16, tag="ch",
                                          name="acc0") if acc is None else \
                            pm_pool.tile([128, QT_W], FP16, tag="pm")
                        b0 = b_pos[kc] * QT_W
                        nc.vector.tensor_mul(pm, p_sb, nm16_sb[:, b0:b0 + QT_W])
                        if acc is None:
                            acc = pm
                        else:
                            t = ch_pool.tile([128, QT_W], FP16, tag="ch")
                            nc.vector.tensor_add(t, acc, pm)
                            acc = t

                    pv_queue.append((vchunk, pm, kc, (b, h)))
                    # uniform software pipeline: one PV pop per kc (the queue
                    # carries the previous half's 6-entry tail across the
                    # boundary, draining at kc0-5, BEFORE the kc6 epilogue
                    # reads that half's o accumulation); in the last half,
                    # drain progressively so the kernel tail is short
                    depth = PV_DEPTH
                    if last_half:
                        depth = max(1, PV_DEPTH - max(0, kc - 8))
                    while len(pv_queue) > depth:
                        pop_pv()

                if last_half:
                    while pv_queue:
                        pop_pv()
                prev_o_ps = o_ps
                pending_epi = (o_ps, acc, accg, ob_sb, h, b)

        if pending_epi is not None:
            emit_epilogue(*pending_epi)

    nc.compile()
    return nc


def _get_nc():
    global _CACHED_NC
    if _CACHED_NC is None:
        _CACHED_NC = _build()
    return _CACHED_NC


def kernel(queries, keys, values, mask_idx, **_unused):
    global LAST_RESULTS
    _ensure_axon_hooks_stub()
    from concourse import bass_utils

    queries = np.asarray(queries, dtype=np.float32)
    keys = np.asarray(keys, dtype=np.float32)
    values = np.asarray(values, dtype=np.float32)
    mask_idx = np.asarray(mask_idx)

    # host-side shard + reformat (layout only; no attention math on host)
    qt = np.ascontiguousarray(
        queries.reshape(N_CORES, BPC, Q, D).transpose(0, 1, 3, 2)).astype(
        np.float16)
    kt = np.ascontiguousarray(
        keys.reshape(N_CORES, BPC, K, D).transpose(0, 1, 3, 2)).astype(
        np.float16)
    v = values.reshape(N_CORES, BPC, K, D).astype(np.float16)
    # keep-mask, transposed to [K, Q] per head, split into the 2 formats
    nmt = np.ascontiguousarray(
        (~mask_idx.astype(bool)).reshape(N_CORES, BPC, Q, K)
        .transpose(0, 1, 3, 2))
    kcs = np.arange(K) // 128
    a_rows = np.isin(kcs, A_KCS)
    nm16 = np.ascontiguousarray(nmt[:, :, ~a_rows, :]).astype(np.float16)
    nm8 = (np.ascontiguousarray(nmt[:, :, a_rows, :]).astype(np.uint8)
           * np.uint8(FP8E4_ONE_BYTE))
    negi8 = (np.eye(128) * FP8E5_LAM_BYTE).astype(np.uint8)

    in_maps = [
        {"qt": qt[c], "kt": kt[c], "v": np.ascontiguousarray(v[c]),
         "nm16": nm16[c], "nm8": nm8[c], "negi8": negi8}
        for c in range(N_CORES)
    ]

    nc = _get_nc()
    res = bass_utils.run_bass_kernel_spmd(nc, in_maps, core_ids=list(range(N_CORES)))
    LAST_RESULTS = res

    # gather + unshard: out is O^T [BPC, d, q] per core -> [B, Q, D]
    ot = np.stack([res.results[c]["out"] for c in range(N_CORES)])
    return np.ascontiguousarray(
        ot.transpose(0, 1, 3, 2).reshape(B, Q, D)).astype(np.float32)
